# revision 5
# baseline (speedup 1.0000x reference)
"""DistanceLoss kernel v4 for Trainium2 (8 NeuronCores, data-parallel over batch).

Computes mean(MARGIN + dist[i, label_i] - min_{c != label_i} dist[i, c]) for
row-normalized WO [N, D] vs class embeddings emb [C, D] via the GEMM identity
d2 = x2 + e2 - 2 x.e.

Design (HW-measured decisions):
- fp16 k=128 matmuls, no perf modes: DoubleRow occupies both PE weight
  buffers so each DR matmul pays ~213ns serialized LDWEIGHTS (v1's stream:
  102us); plain fp16 hides weight loads behind FWL + the background buffer
  (same stream: 75us) and halves quantization noise.
- psum = x.e - |x|*e2/2 with x, e unnormalized fp16; the masked scan's
  imm2=2.0 rescales, so min_d2 = 1 - rnorm * max_scan and -e2/2 never
  overflows fp16.
- eT/aT via PE transposes (f32 -> PSUM) + fp16-cast evacuations at the HEAD
  of the ScalarE queue: the tpp pool has 2 PSUM banks, so the transpose
  pipeline advances at evacuation rate -- evacs must not queue behind
  squares. (xbar DMA-transposes were tried and are catastrophically
  serialized against concurrent SWDGE traffic: +77us.)
- Single fused m-loop: for each row-tile, both 1024-wide psum halves are
  computed back-to-back, the two chained masked scans follow, then that
  tile's label dot. No h-pass boundary, scan tail is one tile deep.
- Per-row |x| rank-1 lhsT rows: v1's [128,1]->[1,128] PE transpose + tiny
  ScalarE f16 copies (ring-latency-free). -e2/2 rows: two batched DRAM
  bounces (partition->free transpose) on the SWDGE ring, which then carries
  the 16 late-offset f32 label-row gathers.
- Label path is fp32/f16 exact-ish: gathered emb[label] rows, DVE
  TENSOR_TENSOR_REDUCE dots against wo f32, ScalarE square-accumulate for
  |emb[label]|^2; GEMM quantization never touches it.

Row-block m holds rows {i : i % 16 == m}; class-block c holds classes
{j : j % 16 == c}: every DMA contiguous per partition; the label's matrix
column = (label & 15) << 7 | label >> 4 in exact int ops.

Sharding: WO/label split over N across 8 cores, emb replicated; mean on host.
_build(repeat=R) wraps the body in a hardware For_i loop (timing harness).
"""

import sys

if "/opt/trn_rl_repo" not in sys.path:
    sys.path.insert(0, "/opt/trn_rl_repo")

import numpy as np

import concourse.bacc as bacc
import concourse.bass as bass
import concourse.mybir as mybir
import concourse.tile as tile
from concourse.bass_utils import run_bass_kernel_spmd
from concourse.dve_ops import TENSOR_MASK_REDUCE, TENSOR_TENSOR_REDUCE
from concourse.masks import make_identity

MARGIN = 1.0
N_CORES = 8
N_FULL, C, D = 16384, 2048, 512
P = 128
NN = N_FULL // N_CORES          # rows per core (2048)
NT = NN // P                    # row tiles per core (16)
CT = C // P                     # class tiles (16)
KT = D // P                     # contraction tiles (4)
HALF = C // 2                   # psum tile width (1024)

f32 = mybir.dt.float32
f16 = mybir.dt.float16
i32 = mybir.dt.int32
Alu = mybir.AluOpType
Act = mybir.ActivationFunctionType

NEG_BIG = -3.0e38
QUAKE = 0x5F3759DF


def _rsqrt(nc, pool, x_ap, w, name, iters=2):
    """1/sqrt(x) on DVE: bit-trick seed + Newton. x_ap: [P, w] f32."""
    si = pool.tile([P, w], i32, tag=f"rs_i{name}")
    nc.vector.tensor_scalar(
        out=si[:], in0=x_ap.bitcast(i32), scalar1=1, scalar2=0,
        op0=Alu.logical_shift_right, op1=Alu.bitwise_not,
    )
    nc.vector.tensor_scalar(out=si[:], in0=si[:], scalar1=QUAKE + 1, scalar2=None,
                            op0=Alu.add)
    y = pool.tile([P, w], f32, tag=f"rs_y{name}")
    nc.vector.tensor_copy(out=y[:], in_=si[:].bitcast(f32))
    t = pool.tile([P, w], f32, tag=f"rs_t{name}")
    for _ in range(iters):
        nc.vector.tensor_mul(out=t[:], in0=y[:], in1=y[:])
        nc.vector.tensor_mul(out=t[:], in0=t[:], in1=x_ap)
        nc.vector.tensor_scalar(out=t[:], in0=t[:], scalar1=-0.5, scalar2=1.5,
                                op0=Alu.mult, op1=Alu.add)
        nc.vector.tensor_mul(out=y[:], in0=y[:], in1=t[:])
    return y


def _build(repeat=1):
    nc = bacc.Bacc("TRN2", target_bir_lowering=False, debug=False)

    wo_d = nc.dram_tensor("WO", [NN, D], f32, kind="ExternalInput")
    emb_d = nc.dram_tensor("emb", [C, D], f32, kind="ExternalInput")
    lab_d = nc.dram_tensor("label", [NN, 1], i32, kind="ExternalInput")
    out_d = nc.dram_tensor("out", [P, NT], f32, kind="ExternalOutput")

    from contextlib import nullcontext

    with tile.TileContext(nc) as tc:
        with (
            tc.tile_pool(name="persist", bufs=1) as pp,
            tc.tile_pool(name="elab", bufs=NT) as elp,
            tc.tile_pool(name="sq", bufs=2) as sqp,
            tc.tile_pool(name="tmp", bufs=8) as tmp_p,
            tc.tile_pool(name="mm", bufs=4, space="PSUM") as mmp,
        ):
            identf = pp.tile([P, P], f32)
            make_identity(nc, identf[:])
            warm_a = pp.tile([P, P], f16)
            warm_b = pp.tile([P, 512], f16)
            nc.vector.memset(warm_a[:], 0.5)
            nc.vector.memset(warm_b[:], 0.5)
            loop_cm = tc.For_i(0, repeat, 1) if repeat > 1 else nullcontext()
            with loop_cm:
                _emit_body(nc, wo_d, emb_d, lab_d, out_d,
                           pp, elp, sqp, tmp_p, mmp, identf,
                           warm_a, warm_b)

    nc.compile()
    return nc


def _emit_body(nc, wo_d, emb_d, lab_d, out_d,
               pp, elp, sqp, tmp_p, mmp, identf, warm_a, warm_b):
    x2 = pp.tile([P, NT], f32)
    rnorm = pp.tile([P, NT], f32)
    e2s_dram = nc.dram_tensor("e2scratch", [1, C], f16)
    eT = pp.tile([P, KT, C], f16)
    aT = pp.tile([P, KT, NN], f16)
    e_all = pp.tile([P, CT, D], f32)
    wo_all = pp.tile([P, NT, D], f32)
    emb_v = emb_d.rearrange("(p c) d -> p c d", c=CT)
    wo_v = wo_d.rearrange("(p t) d -> p t d", t=NT)
    lab_v = lab_d[:, 0].rearrange("(p m) -> p m", m=NT)

    e2c_g = [pp.tile([P, 4], f32, name=f"e2c_{g}") for g in range(4)]
    # -e2/2 halves: [P, 8] written by two groups each, bounced in one pair
    e2h_h = [pp.tile([P, 8], f16, name=f"e2h_{h}") for h in range(2)]
    e2p_h = [pp.tile([1, HALF], f16, name=f"e2p_{h}") for h in range(2)]
    xn_all = pp.tile([P, NT], f32)
    xnT = [None] * NT

    negmax = pp.tile([P, NT], f32)
    acc0 = pp.tile([P, NT], f32)
    dots = pp.tile([P, NT], f32)
    elab2 = pp.tile([P, NT], f32)
    elab_tiles = [None] * NT
    labi = pp.tile([P, NT], i32)       # early copy: label-window math

    labj = pp.tile([P, NT], f32)       # column index of label class
    labf1 = pp.tile([P, NT], f32)      # col + 1
    labh = pp.tile([P, NT], f32)       # col - HALF
    labh1 = pp.tile([P, NT], f32)      # col - HALF + 1

    def load(g):
        sl = slice(g * 4, (g + 1) * 4)
        nc.sync.dma_start(out=e_all[:, sl, :], in_=emb_v[:, sl, :])
        nc.scalar.dma_start(out=wo_all[:, sl, :], in_=wo_v[:, sl, :])

    def label_smalls():
        # matrix column of class L is (L & 15) << 7 | (L >> 4)
        lm = tmp_p.tile([P, NT], i32, tag="lm")
        nc.vector.tensor_scalar(out=lm[:], in0=labi[:], scalar1=15, scalar2=7,
                                op0=Alu.bitwise_and, op1=Alu.logical_shift_left)
        ldt = tmp_p.tile([P, NT], i32, tag="ld")
        nc.vector.tensor_scalar(out=ldt[:], in0=labi[:], scalar1=4, scalar2=None,
                                op0=Alu.logical_shift_right)
        nc.vector.tensor_tensor(out=lm[:], in0=lm[:], in1=ldt[:], op=Alu.bitwise_or)
        nc.vector.tensor_copy(out=labj[:], in_=lm[:])
        nc.vector.tensor_scalar_add(out=labf1[:], in0=labj[:], scalar1=1.0)
        nc.vector.tensor_scalar_add(out=labh[:], in0=labj[:], scalar1=float(-HALF))
        nc.vector.tensor_scalar_add(out=labh1[:], in0=labj[:],
                                    scalar1=float(1 - HALF))

    def ttr_e(g):
        # |e|^2 on DVE (keeps the e2 chain off the ScalarE queue entirely)
        for i, t in enumerate(range(g * 4, (g + 1) * 4)):
            de = tmp_p.tile([P, 1], f32, tag="dmp", name=f"dme_{t}")
            nc.vector._custom_dve(
                TENSOR_TENSOR_REDUCE, out=de[:].broadcast_to([P, D]),
                in0=e_all[:, t, :], in1=e_all[:, t, :], s0=0.0, s1=1.0,
                accum_out=e2c_g[g][:, i : i + 1],
            )

    def sq_wo(g):
        for t in range(g * 4, (g + 1) * 4):
            sw = sqp.tile([P, D], f16, tag="sq", name=f"sqw_{t}")
            nc.scalar.activation(out=sw[:], in_=wo_all[:, t, :], func=Act.Square,
                                 accum_out=x2[:, t : t + 1])

    def e2smalls(g):
        # -e2/2 as f16 into the right half-tile slice
        h, half_i = divmod(g, 2)
        nc.vector.tensor_scalar_mul(
            out=e2h_h[h][:, half_i * 4 : (half_i + 1) * 4],
            in0=e2c_g[g][:], scalar1=-0.5)

    def e2bounce(h):
        # batched -e2/2 bounce for one 1024-class half (SWDGE ring)
        qs = slice(h * HALF, (h + 1) * HALF)
        nc.gpsimd.dma_start(
            out=e2s_dram[0:1, qs].rearrange("o (ct p) -> o p ct", p=P),
            in_=e2h_h[h][:])
        nc.gpsimd.dma_start(out=e2p_h[h][:], in_=e2s_dram[:, qs])

    def rsqrt_d(g):
        sl = slice(g * 4, (g + 1) * 4)
        y = _rsqrt(nc, tmp_p, x2[:, sl], 4, "n", iters=2)
        nc.vector.tensor_scalar_min(out=rnorm[:, sl], in0=y[:], scalar1=1.0e12)
        # |x| = x2 * rsqrt(x2); PE-transpose each column to a [1, P] f16 row
        nc.vector.tensor_mul(out=xn_all[:, sl], in0=x2[:, sl], in1=rnorm[:, sl])
        for m in range(g * 4, (g + 1) * 4):
            tp = mmp.tile([P, HALF], f32, tag="mm", name=f"tpx_{m}")
            nc.tensor.transpose(out=tp[0:1, 0:P], in_=xn_all[:, m : m + 1],
                                identity=identf[:])
            x16 = pp.tile([1, P], f16, name=f"xnT_{m}")
            xnT[m] = x16
            nc.scalar.copy(out=x16[:], in_=tp[0:1, 0:P])

    def tpe(g):
        # PE transpose -> PSUM f32, evacuate with f16 cast on ScalarE
        for cc in range(g * 4, (g + 1) * 4):
            tp = mmp.tile([P, HALF], f32, tag="mm", name=f"tpe_{cc}")
            for k in range(KT):
                nc.tensor.transpose(out=tp[:, k * P : (k + 1) * P],
                                    in_=e_all[:, cc, k * P : (k + 1) * P],
                                    identity=identf[:])
            nc.scalar.copy(
                out=eT[:, :, cc * P : (cc + 1) * P],
                in_=tp[:, 0:512].rearrange("p (k q) -> p k q", k=KT))

    def tpa(g):
        for m in range(g * 4, (g + 1) * 4):
            tp = mmp.tile([P, HALF], f32, tag="mm", name=f"tpa_{m}")
            for k in range(KT):
                nc.tensor.transpose(out=tp[:, k * P : (k + 1) * P],
                                    in_=wo_all[:, m, k * P : (k + 1) * P],
                                    identity=identf[:])
            nc.scalar.copy(
                out=aT[:, :, m * P : (m + 1) * P],
                in_=tp[:, 0:512].rearrange("p (k q) -> p k q", k=KT))

    pm_tiles = {}

    def mm_mms(h, m):
        pm = mmp.tile([P, HALF], f32, tag="mm", name=f"pm_{h}_{m}")
        pm_tiles[(h, m)] = pm
        for k in range(KT):
            for ns in range(2):
                col0 = h * HALF + ns * 512
                nc.tensor.matmul(
                    out=pm[:, ns * 512 : (ns + 1) * 512],
                    lhsT=aT[:, k, m * P : (m + 1) * P],
                    rhs=eT[:, k, col0 : col0 + 512],
                    start=(k == 0), stop=False,
                )
        for ns in range(2):
            nc.tensor.matmul(
                out=pm[:, ns * 512 : (ns + 1) * 512],
                lhsT=xnT[m][:],
                rhs=e2p_h[h][0:1, ns * 512 : (ns + 1) * 512],
                start=False, stop=True,
            )

    def mm_red(h, m):
        # masked max over c != label (inverted single-index window); imm2=2.0
        # recovers 2*x.e - |x|*e2 from psum = x.e - |x|*e2/2
        st_all = labf1 if h == 0 else labh1
        en_all = labj if h == 0 else labh
        pm = pm_tiles[(h, m)]
        dmp = tmp_p.tile([P, 1], f32, tag="dmp", name=f"dmp_{h}_{m}")
        nc.vector._custom_dve(
            TENSOR_MASK_REDUCE,
            out=dmp[:].broadcast_to([P, HALF]),
            in0=pm[:],
            in1=en_all[:, m : m + 1],
            s0=st_all[:, m : m + 1],
            s1=NEG_BIG if h == 0 else acc0[:, m : m + 1],
            imm2=2.0,
            accum_out=(acc0 if h == 0 else negmax)[:, m : m + 1],
        )

    def dot(m):
        dmp2 = tmp_p.tile([P, 1], f32, tag="dmp", name=f"dmpd_{m}")
        nc.vector._custom_dve(
            TENSOR_TENSOR_REDUCE, out=dmp2[:].broadcast_to([P, D]),
            in0=wo_all[:, m, :], in1=elab_tiles[m][:], s0=0.0, s1=1.0,
            accum_out=dots[:, m : m + 1],
        )

    # ---- emission: per-queue order is priority; transposes+evacs lead the
    # PE/Act streams, the e2 chain lives on DVE, bounces+gathers on SWDGE ----
    nc.sync.dma_start(out=labi[:], in_=lab_v)
    for g in range(4):
        load(g)
    label_smalls()

    # fp16 PE warm-up: HAM ramps only on matmul-busy time (PE transposes do
    # not count), so the array would otherwise start the mains cold every
    # trip. Operands are loop-invariant; the result tile is never read.
    warm_ps = mmp.tile([P, HALF], f32, tag="mm", name="warm_ps")
    for _ in range(14):
        nc.tensor.matmul(out=warm_ps[:, 0:512],
                         lhsT=warm_a[:], rhs=warm_b[:],
                         start=True, stop=True)
    tpe(0)
    tpa(0)
    sq_wo(0)
    ttr_e(0)
    e2smalls(0)
    rsqrt_d(0)
    tpe(1)
    tpa(1)
    sq_wo(1)
    ttr_e(1)
    e2smalls(1)
    rsqrt_d(1)
    tpe(2)
    tpa(2)
    sq_wo(2)
    ttr_e(2)
    e2smalls(2)
    e2bounce(0)
    rsqrt_d(2)
    tpe(3)
    tpa(3)
    sq_wo(3)
    ttr_e(3)
    e2smalls(3)
    e2bounce(1)
    rsqrt_d(3)
    # label-row gathers after the bounces on the SWDGE ring; offsets from the
    # late label copy so they post behind the bulk loads
    for m in range(NT):
        gt = elp.tile([P, D], f32, tag="elab", name=f"elab_{m}")
        elab_tiles[m] = gt
        nc.gpsimd.indirect_dma_start(
            out=gt[:], out_offset=None, in_=emb_d[:, :],
            in_offset=bass.IndirectOffsetOnAxis(
                ap=labi[:, m : m + 1], axis=0),
        )
    md2 = tmp_p.tile([P, NT], f32, tag="md2")

    def md2_half(lo, hi):
        # min_{c!=lab} d2 = 1 - rnorm * negmax for columns [lo, hi)
        s = slice(lo, hi)
        nc.vector.tensor_mul(out=md2[:, s], in0=rnorm[:, s], in1=negmax[:, s])
        nc.vector.tensor_scalar(out=md2[:, s], in0=md2[:, s], scalar1=-1.0,
                                scalar2=1.0, op0=Alu.mult, op1=Alu.add)
        nc.vector.tensor_scalar_max(out=md2[:, s], in0=md2[:, s], scalar1=0.0)

    # fused m-loop: both psum halves, chained scans, then the label dot;
    # the min-dist epilogue for rows of m<8 runs mid-loop to shorten the tail
    for m in range(NT):
        mm_mms(0, m)
        mm_mms(1, m)
        mm_red(0, m)
        mm_red(1, m)
        dot(m)
        if m == 8:
            md2_half(0, 8)
    # |emb[label]|^2 on ScalarE; overlaps the back half of the m-loop
    for m in range(NT):
        s = sqp.tile([P, D], f16, tag="sq", name=f"sql_{m}")
        nc.scalar.activation(out=s[:], in_=elab_tiles[m][:], func=Act.Square,
                             accum_out=elab2[:, m : m + 1])

    # ---- epilogue ----
    # label_d2 = 1 + elab2 - 2*rnorm*dot  (x2 of normalized row == 1)
    ld2 = tmp_p.tile([P, NT], f32, tag="ld2")
    nc.vector.tensor_mul(out=ld2[:], in0=rnorm[:], in1=dots[:])
    nc.vector.tensor_scalar(out=ld2[:], in0=ld2[:], scalar1=-2.0, scalar2=1.0,
                            op0=Alu.mult, op1=Alu.add)
    nc.vector.tensor_add(out=ld2[:], in0=ld2[:], in1=elab2[:])
    nc.vector.tensor_scalar_max(out=ld2[:], in0=ld2[:], scalar1=0.0)
    md2_half(8, NT)

    # sqrt(x) = x * rsqrt(x); out = sqrt(ld2) - sqrt(md2)
    rl = _rsqrt(nc, tmp_p, ld2[:], NT, "l", iters=2)
    rm = _rsqrt(nc, tmp_p, md2[:], NT, "m", iters=2)
    nc.vector.tensor_mul(out=rl[:], in0=rl[:], in1=ld2[:])
    nc.vector.tensor_mul(out=rm[:], in0=rm[:], in1=md2[:])
    outv = pp.tile([P, NT], f32)
    nc.vector.tensor_sub(out=outv[:], in0=rl[:], in1=rm[:])
    nc.sync.dma_start(out=out_d[:, :], in_=outv[:])


_NC = None


def kernel(WO, emb_weight, label):
    global _NC
    if _NC is None:
        _NC = _build()

    WO = np.ascontiguousarray(np.asarray(WO, dtype=np.float32))
    emb = np.ascontiguousarray(np.asarray(emb_weight, dtype=np.float32))
    lab = np.asarray(label).astype(np.int32).reshape(N_FULL, 1)

    in_maps = []
    for i in range(N_CORES):
        sl = slice(i * NN, (i + 1) * NN)
        in_maps.append({
            "WO": WO[sl],
            "emb": emb,
            "label": np.ascontiguousarray(lab[sl]),
        })
    res = run_bass_kernel_spmd(_NC, in_maps, core_ids=list(range(N_CORES)))
    vals = np.stack([res.results[i]["out"] for i in range(N_CORES)])
    return np.float32(MARGIN + np.mean(vals.astype(np.float64)))
